# revision 14
# baseline (speedup 1.0000x reference)
"""Trainium2 Bass kernel for nn_Attention_88785563943675.

Single-head attention (the reference reuses identical per-head weights, so
all 4 heads compute the same [B,S,h] output; the concat+WO projection
collapses to a single [h,D] projection with WO_eff = sum of WO row blocks).

Math per batch b:
    Qp = q[b] @ WQ            [S, 50]
    Kp = k[b] @ WK            [S, 50]
    Vp = v[b] @ WV            [S, 50]
    A  = softmax(Qp Kp^T / sqrt(50))   row-wise over k-index
    O  = A @ Vp               [S, 50]
    Y  = O @ WO_eff           [S, 200]

Sharding: 8 cores = (batch b in 0..3) x (query half h in 0..1).

v2 design notes (from baseline NTFF analysis):
  - inputs are cast to bf16 AND d-padded to 256 on the HOST; the d->partition
    transposes of q/k/v are done by the DMA XBAR (dma_start(transpose=True),
    16x128 tiles, ~14ns/tile) during the load -- zero PE/DVE transpose work.
    The baseline spent ~45us of TensorE on 160 LDWEIGHTS-heavy PE transposes.
  - Vp (natural [k,51] layout, AV stationary) is produced by projecting to
    VpT [50, S] with weight-stationary 512-wide matmuls, then XBAR-transposing
    SBUF->SBUF per 512-tile. Row 50 of VpT is preset to 1.0 so the AV matmul
    emits the softmax denominator l as output row 50 (baseline trick).
  - main loop is software-pipelined: per unit u=(half,kb) the PE stream is
    [filler][st(u+1)][av(u)], so the PE never sits in-order behind exp(u) --
    the baseline stalled ~600ns/unit there, which also dropped the PE p-state
    from 2.4GHz to 1.2GHz (measured: phase A matmuls streamed at ~0.42ns/row,
    main loop at ~0.83ns/row).
  - ScalarE runs ONLY the 64 exp activations (PSUM f32 -> SBUF bf16);
    evacuations and the epilogue scaling live on DVE.
  - k/v projection tiles are interleaved into half-0 units as PE filler;
    half-0's output projection is interleaved into half-1.
  - epilogue: Yu = [O_unnorm | l] @ rhs_aug (f32r, 256-wide => 1 cyc/row),
    rows scaled by 1/l via DVE reciprocal + tensor_scalar_mul, DMA out.
"""

import math

import numpy as np

import concourse.bacc as bacc
import concourse.bass as bass
import concourse.mybir as mybir
import concourse.tile as tile
from concourse.bass_utils import run_bass_kernel_spmd
from concourse.masks import make_identity

B = 4
S = 4096
D = 200
DP = 256  # host-padded d (multiple of 128 for the DMA XBAR)
E = 50  # size per head
N_CORES = 8
SQ = S // 2  # q rows per core
SK = S  # k rows per core
SCALE = 1.0 / math.sqrt(E)

F32 = mybir.dt.float32
F32R = mybir.dt.float32r
BF16 = mybir.dt.bfloat16

N_KB = SK // 128  # 32 k-blocks
N_KT = SK // 512  # 8 k/v projection tiles
N_QT = SQ // 512  # 4 q projection tiles


def _emit(nc, tc, q_ap, k_ap, v_ap, wq_ap, wk_ap, wv_ap, wo_ap, out_ap):
    import contextlib

    stack = contextlib.ExitStack()
    singles = stack.enter_context(tc.tile_pool(name="singles", bufs=1))

    ident = singles.tile([128, 128], BF16)
    make_identity(nc, ident)

    # Weights: DRAM [256, 50] f32 -> SBUF [128, 2, 50] -> bf16. Issued on the
    # gpsimd (SWDGE) queue so they don't serialize against the input XBAR
    # transposes on the sync queue.
    w_bf = {}
    for name, ap in (("wq", wq_ap), ("wk", wk_ap), ("wv", wv_ap)):
        wf = singles.tile([128, 2, E], F32, tag=f"{name}_f32")
        nc.gpsimd.dma_start(out=wf, in_=ap.rearrange("(c p) e -> p c e", c=2))
        wb = singles.tile([128, 2, E], BF16, tag=f"{name}_bf16")
        nc.vector.tensor_copy(out=wb, in_=wf)
        w_bf[name] = wb

    # Output-projection rhs [51, 256]: rows 0:50 cols 0:200 = WO_eff,
    # row 50 col 200 = 1.0 (passes the softmax denominator l through).
    rhs_stage = singles.tile([E + 1, 256], F32)
    nc.vector.memset(rhs_stage, 0.0)
    nc.gpsimd.dma_start(out=rhs_stage[0:E, 0:D], in_=wo_ap)
    nc.vector.memset(rhs_stage[:, 200:201], 1.0)
    nc.vector.memset(rhs_stage[0:E, 200:201], 0.0)
    rhs_aug = singles.tile([E + 1, 256], F32R)
    nc.vector.tensor_copy(out=rhs_aug, in_=rhs_stage)

    # Transposed inputs (filled by the DMA XBAR), chunk c = d in [128c,128c+128)
    qT = singles.tile([128, 2, SQ], BF16)
    kT = singles.tile([128, 2, SK], BF16)
    vT = singles.tile([128, 2, SK], BF16)

    # Projected tensors
    KpT = singles.tile([E, SK], BF16)  # [50, 4096]
    QpT = singles.tile([E, SQ], BF16)  # [50, 2048]
    VpTp = singles.tile([64, SK], BF16)  # rows 0:50 VpT, rows 50:64 zero
    nc.vector.memset(VpTp[32:64, :], 0.0)  # partition starts must be 32-aligned
    Vp = singles.tile([128, N_KB, 64], BF16)  # XBAR of VpTp; [:, kb, 0:51] used
    OT = singles.tile([E + 1, SQ], F32R)  # [51, 2048] O^T unnormalized + l

    # Input XBAR loads, split into s-quarters for pipelining. The XBAR
    # executes serially, so issue in critical-path order: q (unblocks QpT +
    # first scores), k-s0 (KpT tile 0), v-s0, then alternate k/v quarters.
    nc.sync.dma_start(out=qT[:, 0, :], in_=q_ap[:, 0:128], transpose=True)
    nc.sync.dma_start(out=qT[:, 1, :], in_=q_ap[:, 128:256], transpose=True)
    for sq in range(4):
        s0, s1 = sq * 1024, (sq + 1) * 1024
        for name, xT, x_ap in (("k", kT, k_ap), ("v", vT, v_ap)):
            for c in range(2):
                nc.sync.dma_start(
                    out=xT[:, c, s0:s1],
                    in_=x_ap[s0:s1, c * 128 : (c + 1) * 128],
                    transpose=True,
                )

    import contextlib as _ctx

    pt_pool = stack.enter_context(tc.tile_pool(name="pt", bufs=4))
    yo_pool = stack.enter_context(tc.tile_pool(name="yo", bufs=3))
    rec_pool = stack.enter_context(tc.tile_pool(name="rec", bufs=3))
    main_stack = _ctx.ExitStack()
    st_psum = main_stack.enter_context(tc.tile_pool(name="st_ps", bufs=2, space="PSUM"))
    ot_psum = main_stack.enter_context(tc.tile_pool(name="ot_ps", bufs=2, space="PSUM"))
    pj_psum = main_stack.enter_context(tc.tile_pool(name="pj_ps", bufs=1, space="PSUM"))
    yu_psum = main_stack.enter_context(tc.tile_pool(name="yu_ps", bufs=1, space="PSUM"))

    # PE warm-up: the TensorE takes ~10us to execute its first instruction
    # after becoming ready; soak that up during the DMA ramp.
    warm = yu_psum.tile([128, 256], F32, tag="yu", name="warm").bitcast(BF16)
    nc.tensor.transpose(out=warm[0:1, 0:128], in_=ident[:, 0:1], identity=ident)

    def proj_tile(dest_name, t):
        """One 512-wide projection tile: KpT/QpT/VpT[:, 512t:512(t+1)]."""
        wname, xT, dest = {
            "k": ("wk", kT, KpT),
            "q": ("wq", qT, QpT),
            "v": ("wv", vT, VpTp),
        }[dest_name]
        s0, s1 = t * 512, (t + 1) * 512
        pj = pj_psum.tile([E, 512], F32, tag="pj")
        for c in range(2):
            nc.tensor.matmul(
                pj, lhsT=w_bf[wname][:, c, :], rhs=xT[:, c, s0:s1],
                start=(c == 0), stop=(c == 1),
            )
        nc.vector.tensor_copy(out=dest[0:E, s0:s1], in_=pj)
        if dest_name == "v":
            # natural-layout Vp block for the AV stationary via SBUF XBAR;
            # column 50 then becomes the all-ones column (emits the softmax
            # denominator l as AV output row 50)
            nc.sync.dma_start(
                out=Vp[:, 4 * t : 4 * t + 4, :], in_=VpTp[:, s0:s1],
                transpose=True,
            )
            nc.vector.memset(Vp[:, 4 * t : 4 * t + 4, E : E + 1], 1.0)

    def emit_st(u):
        half, kb = divmod(u, N_KB)
        st = st_psum.tile([128, 1024], F32, tag="st")
        for sub in range(2):
            nc.tensor.matmul(
                st[:, sub * 512 : (sub + 1) * 512],
                lhsT=KpT[:, kb * 128 : (kb + 1) * 128],
                rhs=QpT[:, half * 1024 + sub * 512 : half * 1024 + (sub + 1) * 512],
                start=True, stop=True,
            )
        return st

    def emit_epilogue_qb(qb, pool):
        yu = pool.tile([128, 256], F32, tag="yu", name=f"yu{qb}")
        nc.tensor.matmul(
            yu, lhsT=OT[:, qb * 128 : (qb + 1) * 128], rhs=rhs_aug,
            start=True, stop=True,
        )
        rec = rec_pool.tile([128, 1], F32, tag="rec")
        nc.vector.reciprocal(rec, yu[:, 200:201])
        yo = yo_pool.tile([128, D], F32, tag="yo")
        nc.vector.tensor_scalar_mul(yo, yu[:, 0:D], rec)
        nc.sync.dma_start(out=out_ap[qb * 128 : (qb + 1) * 128, :], in_=yo)

    # ---- Prologue: minimum work before the first score matmul --------------
    proj_tile("q", 0)
    proj_tile("q", 1)
    proj_tile("k", 0)
    proj_tile("v", 0)

    # filler schedule: half-0 unit u -> k/v/q projection tiles (K_t needed by
    # unit 4t-2, V_t by unit 4t, q tiles 2-3 by unit 31); half-1 units get
    # half-0's epilogue blocks.
    fillers = {}
    for t in range(1, N_KT):
        fillers[2 * (t - 1)] = ("k", t)
        fillers[2 * (t - 1) + 1] = ("v", t)
    fillers[14] = ("q", 2)
    fillers[15] = ("q", 3)

    st_tiles = {0: emit_st(0)}
    ot_tiles = {}

    for u in range(2 * N_KB):
        half, kb = divmod(u, N_KB)
        if kb == 0:
            ot_tiles[half] = [
                ot_psum.tile([E + 1, 512], F32, tag="ot", name=f"ot{half}_{i}")
                for i in range(2)
            ]
        if u == N_KB:
            # evacuate half-0's O accumulators so their PSUM slots rotate
            for qsub in range(2):
                nc.vector.tensor_copy(
                    out=OT[:, qsub * 512 : (qsub + 1) * 512],
                    in_=ot_tiles[0][qsub],
                )
        # PE filler work for this unit
        if half == 0:
            f = fillers.get(kb)
            if f is not None:
                proj_tile(*f)
        else:
            if kb >= 1 and (kb - 1) % 2 == 0 and (kb - 1) // 2 < 8:
                emit_epilogue_qb((kb - 1) // 2, yu_psum)
        # next unit's scores (keeps PE busy while ScalarE runs exp(u))
        if u + 1 < 2 * N_KB:
            st_tiles[u + 1] = emit_st(u + 1)
        # exp(u)
        st = st_tiles.pop(u)
        pt = pt_pool.tile([128, 1024], BF16, tag="pt")
        nc.scalar.activation(
            out=pt, in_=st, func=mybir.ActivationFunctionType.Exp, scale=SCALE
        )
        # AV(u)
        for qsub in range(2):
            nc.tensor.matmul(
                ot_tiles[half][qsub][0 : E + 1, :],
                lhsT=Vp[:, kb, 0 : E + 1],
                rhs=pt[:, qsub * 512 : (qsub + 1) * 512],
                start=(kb == 0), stop=(kb == N_KB - 1),
            )

    # ---- Tail: evacuate half-1, remaining epilogue -------------------------
    for qsub in range(2):
        nc.vector.tensor_copy(
            out=OT[:, 1024 + qsub * 512 : 1024 + (qsub + 1) * 512],
            in_=ot_tiles[1][qsub],
        )
    # close the main-loop PSUM pools so the tail epilogue can quadruple-buffer
    main_stack.close()
    tail_psum = stack.enter_context(tc.tile_pool(name="tail_ps", bufs=4, space="PSUM"))
    for qb in range(8, 16):
        emit_epilogue_qb(qb, tail_psum)

    stack.close()


_NC_CACHE = None


def build_nc():
    global _NC_CACHE
    if _NC_CACHE is not None:
        return _NC_CACHE
    nc = bacc.Bacc(
        "TRN2", target_bir_lowering=False, debug=False, num_devices=N_CORES
    )
    q_ap = nc.dram_tensor("q", [SQ, DP], BF16, kind="ExternalInput").ap()
    k_ap = nc.dram_tensor("k", [SK, DP], BF16, kind="ExternalInput").ap()
    v_ap = nc.dram_tensor("v", [SK, DP], BF16, kind="ExternalInput").ap()
    wq_ap = nc.dram_tensor("wq", [DP, E], F32, kind="ExternalInput").ap()
    wk_ap = nc.dram_tensor("wk", [DP, E], F32, kind="ExternalInput").ap()
    wv_ap = nc.dram_tensor("wv", [DP, E], F32, kind="ExternalInput").ap()
    wo_ap = nc.dram_tensor("wo", [E, D], F32, kind="ExternalInput").ap()
    out_ap = nc.dram_tensor("out", [SQ, D], F32, kind="ExternalOutput").ap()

    with tile.TileContext(nc) as tc:
        _emit(nc, tc, q_ap, k_ap, v_ap, wq_ap, wk_ap, wv_ap, wo_ap, out_ap)
    nc.compile()
    _NC_CACHE = nc
    return nc


def make_in_maps(q, k, v, WQ, WK, WV, WO):
    import ml_dtypes

    bf16 = ml_dtypes.bfloat16

    def padcast(x):
        x = np.asarray(x, np.float32)
        out = np.zeros(x.shape[:-1] + (DP,), dtype=bf16)
        out[..., :D] = x.astype(bf16)
        return out

    qb, kb_, vb = padcast(q), padcast(k), padcast(v)

    def wpad(w):
        w = np.asarray(w, np.float32)
        out = np.zeros((DP, E), np.float32)
        out[:D, :] = w
        return out

    WQp, WKp, WVp = wpad(WQ), wpad(WK), wpad(WV)
    WO = np.asarray(WO, np.float32)
    # All 4 heads share WQ/WK/WV, so concat+WO == O @ (sum of WO blocks)
    wo_eff = WO.reshape(4, E, D).sum(axis=0).astype(np.float32)
    in_maps = []
    for c in range(N_CORES):
        b, h = c // 2, c % 2
        in_maps.append(
            {
                "q": np.ascontiguousarray(qb[b, h * SQ : (h + 1) * SQ, :]),
                "k": np.ascontiguousarray(kb_[b]),
                "v": np.ascontiguousarray(vb[b]),
                "wq": WQp, "wk": WKp, "wv": WVp, "wo": wo_eff,
            }
        )
    return in_maps


def assemble(results):
    out = np.empty((B, S, D), np.float32)
    for c in range(N_CORES):
        b, h = c // 2, c % 2
        out[b, h * SQ : (h + 1) * SQ, :] = results[c]["out"]
    return out


def kernel(q, k, v, WQ, WK, WV, WO):
    nc = build_nc()
    in_maps = make_in_maps(q, k, v, WQ, WK, WV, WO)
    res = run_bass_kernel_spmd(nc, in_maps, core_ids=list(range(N_CORES)))
    return assemble(res.results)


if __name__ == "__main__":
    # quick self-run with random data
    rng = np.random.default_rng(0)
    q = rng.standard_normal((B, S, D)).astype(np.float32)
    k = rng.standard_normal((B, S, D)).astype(np.float32)
    v = rng.standard_normal((B, S, D)).astype(np.float32)
    WQ = rng.standard_normal((D, E)).astype(np.float32) * 0.08
    WK = rng.standard_normal((D, E)).astype(np.float32) * 0.08
    WV = rng.standard_normal((D, E)).astype(np.float32) * 0.08
    WO = rng.standard_normal((4 * E, D)).astype(np.float32) * 0.08
    out = kernel(q, k, v, WQ, WK, WV, WO)
    print("out", out.shape, out.dtype, np.abs(out).mean())


# revision 18
# speedup vs baseline: 1.1298x; 1.1298x over previous
"""Trainium2 Bass kernel for nn_Attention_88785563943675.

Single-head attention (the reference reuses identical per-head weights, so
all 4 heads compute the same [B,S,h] output; the concat+WO projection
collapses to a single [h,D] projection with WO_eff = sum of WO row blocks).

Math per batch b:
    Qp = q[b] @ WQ            [S, 50]
    Kp = k[b] @ WK            [S, 50]
    Vp = v[b] @ WV            [S, 50]
    A  = softmax(Qp Kp^T / sqrt(50))   row-wise over k-index
    O  = A @ Vp               [S, 50]
    Y  = O @ WO_eff           [S, 200]

Sharding: 8 cores = (batch b in 0..3) x (query half h in 0..1).

v2 design notes (from baseline NTFF analysis):
  - inputs are cast to bf16 AND d-padded to 256 on the HOST; the d->partition
    transposes of q/k/v are done by the DMA XBAR (dma_start(transpose=True),
    16x128 tiles, ~14ns/tile) during the load -- zero PE/DVE transpose work.
    The baseline spent ~45us of TensorE on 160 LDWEIGHTS-heavy PE transposes.
  - Vp (natural [k,51] layout, AV stationary) is produced by projecting to
    VpT [50, S] with weight-stationary 512-wide matmuls, then XBAR-transposing
    SBUF->SBUF per 512-tile. Row 50 of VpT is preset to 1.0 so the AV matmul
    emits the softmax denominator l as output row 50 (baseline trick).
  - main loop is software-pipelined: per unit u=(half,kb) the PE stream is
    [filler][st(u+1)][av(u)], so the PE never sits in-order behind exp(u) --
    the baseline stalled ~600ns/unit there, which also dropped the PE p-state
    from 2.4GHz to 1.2GHz (measured: phase A matmuls streamed at ~0.42ns/row,
    main loop at ~0.83ns/row).
  - ScalarE runs ONLY the 64 exp activations (PSUM f32 -> SBUF bf16);
    evacuations and the epilogue scaling live on DVE.
  - k/v projection tiles are interleaved into half-0 units as PE filler;
    half-0's output projection is interleaved into half-1.
  - epilogue: Yu = [O_unnorm | l] @ rhs_aug (f32r, 256-wide => 1 cyc/row),
    rows scaled by 1/l via DVE reciprocal + tensor_scalar_mul, DMA out.
"""

import math

import numpy as np

import concourse.bacc as bacc
import concourse.bass as bass
import concourse.mybir as mybir
import concourse.tile as tile
from concourse.bass_utils import run_bass_kernel_spmd
from concourse.masks import make_identity

B = 4
S = 4096
D = 200
DP = 256  # host-padded d (multiple of 128 for the DMA XBAR)
E = 50  # size per head
N_CORES = 8
SQ = S // 2  # q rows per core
SK = S  # k rows per core
SCALE = 1.0 / math.sqrt(E)

F32 = mybir.dt.float32
F32R = mybir.dt.float32r
BF16 = mybir.dt.bfloat16

N_KB = SK // 128  # 32 k-blocks
N_KT = SK // 512  # 8 k/v projection tiles
N_QT = SQ // 512  # 4 q projection tiles


def _emit(nc, tc, q_ap, k_ap, v_ap, wq_ap, wk_ap, wv_ap, wo_ap, out_ap):
    import contextlib

    stack = contextlib.ExitStack()
    singles = stack.enter_context(tc.tile_pool(name="singles", bufs=1))

    ident = singles.tile([128, 128], BF16)
    make_identity(nc, ident)

    # Weights: DRAM [256, 50] f32 -> SBUF [128, 2, 50] -> bf16. All DMAs
    # execute serially in EMISSION order (the tile scheduler chains them with
    # cross-queue semaphores), so emission order here is the true DMA
    # schedule: weights first, then input XBARs in need-order.
    w_bf = {}
    for name, ap in (("wq", wq_ap), ("wk", wk_ap), ("wv", wv_ap)):
        wf = singles.tile([128, 2, E], F32, tag=f"{name}_f32")
        nc.sync.dma_start(out=wf, in_=ap.rearrange("(c p) e -> p c e", c=2))
        wb = singles.tile([128, 2, E], BF16, tag=f"{name}_bf16")
        nc.vector.tensor_copy(out=wb, in_=wf)
        w_bf[name] = wb

    # Output-projection rhs [51, 256]: rows 0:50 cols 0:200 = WO_eff,
    # row 50 col 200 = 1.0 (passes the softmax denominator l through).
    rhs_stage = singles.tile([E + 1, 256], F32)
    nc.vector.memset(rhs_stage, 0.0)
    nc.sync.dma_start(out=rhs_stage[0:E, 0:D], in_=wo_ap)
    nc.vector.memset(rhs_stage[:, 200:201], 1.0)
    nc.vector.memset(rhs_stage[0:E, 200:201], 0.0)
    rhs_aug = singles.tile([E + 1, 256], F32R)
    nc.vector.tensor_copy(out=rhs_aug, in_=rhs_stage)

    # Transposed inputs (filled by the DMA XBAR), chunk c = d in [128c,128c+128)
    qT = singles.tile([128, 2, SQ], BF16)
    kT = singles.tile([128, 2, SK], BF16)
    vT = singles.tile([128, 2, SK], BF16)

    # Projected tensors
    KpT = singles.tile([E, SK], BF16)  # [50, 4096]
    QpT = singles.tile([E, SQ], BF16)  # [50, 2048]
    VpTp = singles.tile([64, SK], BF16)  # rows 0:50 VpT, rows 50:64 zero
    nc.vector.memset(VpTp[32:64, :], 0.0)  # partition starts must be 32-aligned
    Vp = singles.tile([128, N_KB, 64], BF16)  # XBAR of VpTp; [:, kb, 0:51] used
    OT = singles.tile([E + 1, SQ], F32R)  # [51, 2048] O^T unnormalized + l

    # Input XBAR transpose loads, emitted piecewise in need-order (q/k/v
    # s-halves or s-quarters). The XBAR runs serially at ~10ns per 16x128
    # tile, so pieces later in the chain arrive later.
    def xbar_in(xT, x_ap, s0, s1):
        for c in range(2):
            nc.sync.dma_start(
                out=xT[:, c, s0:s1],
                in_=x_ap[s0:s1, c * 128 : (c + 1) * 128],
                transpose=True,
            )

    # prologue-critical pieces: q s-half 0, k quarter 0, v quarter 0
    xbar_in(qT, q_ap, 0, 1024)
    xbar_in(kT, k_ap, 0, 1024)
    xbar_in(vT, v_ap, 0, 1024)

    import contextlib as _ctx

    pt_pool = stack.enter_context(tc.tile_pool(name="pt", bufs=4))
    yo_pool = stack.enter_context(tc.tile_pool(name="yo", bufs=3))
    rec_pool = stack.enter_context(tc.tile_pool(name="rec", bufs=3))
    main_stack = _ctx.ExitStack()
    st_psum = main_stack.enter_context(tc.tile_pool(name="st_ps", bufs=2, space="PSUM"))
    ot_psum = main_stack.enter_context(tc.tile_pool(name="ot_ps", bufs=2, space="PSUM"))
    pj_psum = main_stack.enter_context(tc.tile_pool(name="pj_ps", bufs=1, space="PSUM"))
    yu_psum = main_stack.enter_context(tc.tile_pool(name="yu_ps", bufs=1, space="PSUM"))

    # PE warm-up: the TensorE takes ~10us to execute its first instruction
    # after becoming ready; soak that up during the DMA ramp.
    warm = yu_psum.tile([128, 256], F32, tag="yu", name="warm").bitcast(BF16)
    nc.tensor.transpose(out=warm[0:1, 0:128], in_=ident[:, 0:1], identity=ident)

    def proj_tile(dest_name, t):
        """One 512-wide projection tile: KpT/QpT/VpT[:, 512t:512(t+1)]."""
        wname, xT, dest = {
            "k": ("wk", kT, KpT),
            "q": ("wq", qT, QpT),
            "v": ("wv", vT, VpTp),
        }[dest_name]
        s0, s1 = t * 512, (t + 1) * 512
        pj = pj_psum.tile([E, 512], F32, tag="pj")
        for c in range(2):
            nc.tensor.matmul(
                pj, lhsT=w_bf[wname][:, c, :], rhs=xT[:, c, s0:s1],
                start=(c == 0), stop=(c == 1),
            )
        nc.vector.tensor_copy(out=dest[0:E, s0:s1], in_=pj)

    def vp_xbar(t0v, t1v):
        # natural-layout Vp blocks for the AV stationary via SBUF XBAR;
        # column 50 then becomes the all-ones column (emits the softmax
        # denominator l as AV output row 50)
        nc.sync.dma_start(
            out=Vp[:, 4 * t0v : 4 * t1v, :],
            in_=VpTp[:, t0v * 512 : t1v * 512],
            transpose=True,
        )
        nc.vector.memset(Vp[:, 4 * t0v : 4 * t1v, E : E + 1], 1.0)

    def emit_st(u):
        half, kb = divmod(u, N_KB)
        st = st_psum.tile([128, 1024], F32, tag="st")
        for sub in range(2):
            nc.tensor.matmul(
                st[:, sub * 512 : (sub + 1) * 512],
                lhsT=KpT[:, kb * 128 : (kb + 1) * 128],
                rhs=QpT[:, half * 1024 + sub * 512 : half * 1024 + (sub + 1) * 512],
                start=True, stop=True,
            )
        return st

    def emit_epilogue_qb(qb, pool):
        yu = pool.tile([128, 256], F32, tag="yu", name=f"yu{qb}")
        nc.tensor.matmul(
            yu, lhsT=OT[:, qb * 128 : (qb + 1) * 128], rhs=rhs_aug,
            start=True, stop=True,
        )
        rec = rec_pool.tile([128, 1], F32, tag="rec")
        nc.vector.reciprocal(rec, yu[:, 200:201])
        yo = yo_pool.tile([128, D], F32, tag="yo")
        nc.vector.tensor_scalar_mul(yo, yu[:, 0:D], rec)
        nc.sync.dma_start(out=out_ap[qb * 128 : (qb + 1) * 128, :], in_=yo)

    # ---- Prologue: minimum work before the first score matmul --------------
    proj_tile("q", 0)
    proj_tile("q", 1)
    proj_tile("k", 0)
    st_tiles = {0: emit_st(0)}
    proj_tile("v", 0)
    proj_tile("v", 1)
    vp_xbar(0, 2)

    # Per-unit action schedule. PE filler projections are placed after their
    # XBAR piece is due to arrive and before their consumer; DMA pieces are
    # emitted just-in-time so the serial DMA chain matches need-order.
    # KpT tile t is needed when st(4t) is emitted (unit 4t-1); Vp tile t when
    # av(4t) runs (unit 4t); QpT tiles 2-3 when st(32) is emitted (unit 31).
    sched = {
        0: [("k", 1)],
        2: [("dma_kv", "k", 1)],
        3: [("dma_kv", "v", 1)],
        4: [("k", 2)],
        5: [("k", 3)],
        6: [("v", 2)],
        7: [("v", 3), ("vpx", 2, 4)],
        8: [("dma_kv", "k", 2)],
        9: [("dma_kv", "v", 2)],
        10: [("k", 4)],
        11: [("k", 5)],
        12: [("v", 4)],
        13: [("v", 5), ("vpx", 4, 6)],
        14: [("dma_kv", "k", 3)],
        15: [("dma_q",)],
        16: [("dma_kv", "v", 3)],
        17: [("k", 6)],
        18: [("k", 7)],
        19: [("v", 6)],
        20: [("v", 7), ("vpx", 6, 8)],
        21: [("q", 2)],
        22: [("q", 3)],
    }
    for j in range(8):  # half-0 epilogue interleaved into half-1
        sched[N_KB + 1 + 2 * j] = [("epi", j)]

    def run_action(a):
        if a[0] == "dma_kv":
            xT, x_ap = (kT, k_ap) if a[1] == "k" else (vT, v_ap)
            xbar_in(xT, x_ap, a[2] * 1024, (a[2] + 1) * 1024)
        elif a[0] == "dma_q":
            xbar_in(qT, q_ap, 1024, 2048)
        elif a[0] == "vpx":
            vp_xbar(a[1], a[2])
        elif a[0] == "epi":
            emit_epilogue_qb(a[1], yu_psum)
        else:
            proj_tile(a[0], a[1])

    ot_tiles = {}
    for u in range(2 * N_KB):
        half, kb = divmod(u, N_KB)
        if kb == 0:
            ot_tiles[half] = [
                ot_psum.tile([E + 1, 512], F32, tag="ot", name=f"ot{half}_{i}")
                for i in range(2)
            ]
        if u == N_KB:
            # evacuate half-0's O accumulators so their PSUM slots rotate
            for qsub in range(2):
                nc.vector.tensor_copy(
                    out=OT[:, qsub * 512 : (qsub + 1) * 512],
                    in_=ot_tiles[0][qsub],
                )
        # next unit's scores first (keeps PE busy while ScalarE runs exp(u))
        if u + 1 < 2 * N_KB:
            st_tiles[u + 1] = emit_st(u + 1)
        # exp(u)
        st = st_tiles.pop(u)
        pt = pt_pool.tile([128, 1024], BF16, tag="pt")
        nc.scalar.activation(
            out=pt, in_=st, func=mybir.ActivationFunctionType.Exp, scale=SCALE
        )
        # filler work for this unit (fills PE slack while exp(u) runs)
        for a in sched.get(u, ()):
            run_action(a)
        # AV(u)
        for qsub in range(2):
            nc.tensor.matmul(
                ot_tiles[half][qsub][0 : E + 1, :],
                lhsT=Vp[:, kb, 0 : E + 1],
                rhs=pt[:, qsub * 512 : (qsub + 1) * 512],
                start=(kb == 0), stop=(kb == N_KB - 1),
            )

    # ---- Tail: evacuate half-1, remaining epilogue -------------------------
    for qsub in range(2):
        nc.vector.tensor_copy(
            out=OT[:, 1024 + qsub * 512 : 1024 + (qsub + 1) * 512],
            in_=ot_tiles[1][qsub],
        )
    # close the main-loop PSUM pools so the tail epilogue can quadruple-buffer
    main_stack.close()
    tail_psum = stack.enter_context(tc.tile_pool(name="tail_ps", bufs=4, space="PSUM"))
    for qb in range(8, 16):
        emit_epilogue_qb(qb, tail_psum)

    stack.close()


_NC_CACHE = None


def build_nc():
    global _NC_CACHE
    if _NC_CACHE is not None:
        return _NC_CACHE
    nc = bacc.Bacc(
        "TRN2", target_bir_lowering=False, debug=False, num_devices=N_CORES
    )
    q_ap = nc.dram_tensor("q", [SQ, DP], BF16, kind="ExternalInput").ap()
    k_ap = nc.dram_tensor("k", [SK, DP], BF16, kind="ExternalInput").ap()
    v_ap = nc.dram_tensor("v", [SK, DP], BF16, kind="ExternalInput").ap()
    wq_ap = nc.dram_tensor("wq", [DP, E], F32, kind="ExternalInput").ap()
    wk_ap = nc.dram_tensor("wk", [DP, E], F32, kind="ExternalInput").ap()
    wv_ap = nc.dram_tensor("wv", [DP, E], F32, kind="ExternalInput").ap()
    wo_ap = nc.dram_tensor("wo", [E, D], F32, kind="ExternalInput").ap()
    out_ap = nc.dram_tensor("out", [SQ, D], F32, kind="ExternalOutput").ap()

    with tile.TileContext(nc) as tc:
        _emit(nc, tc, q_ap, k_ap, v_ap, wq_ap, wk_ap, wv_ap, wo_ap, out_ap)
    nc.compile()
    _NC_CACHE = nc
    return nc


def make_in_maps(q, k, v, WQ, WK, WV, WO):
    import ml_dtypes

    bf16 = ml_dtypes.bfloat16

    def padcast(x):
        x = np.asarray(x, np.float32)
        out = np.zeros(x.shape[:-1] + (DP,), dtype=bf16)
        out[..., :D] = x.astype(bf16)
        return out

    qb, kb_, vb = padcast(q), padcast(k), padcast(v)

    def wpad(w):
        w = np.asarray(w, np.float32)
        out = np.zeros((DP, E), np.float32)
        out[:D, :] = w
        return out

    WQp, WKp, WVp = wpad(WQ), wpad(WK), wpad(WV)
    WO = np.asarray(WO, np.float32)
    # All 4 heads share WQ/WK/WV, so concat+WO == O @ (sum of WO blocks)
    wo_eff = WO.reshape(4, E, D).sum(axis=0).astype(np.float32)
    in_maps = []
    for c in range(N_CORES):
        b, h = c // 2, c % 2
        in_maps.append(
            {
                "q": np.ascontiguousarray(qb[b, h * SQ : (h + 1) * SQ, :]),
                "k": np.ascontiguousarray(kb_[b]),
                "v": np.ascontiguousarray(vb[b]),
                "wq": WQp, "wk": WKp, "wv": WVp, "wo": wo_eff,
            }
        )
    return in_maps


def assemble(results):
    out = np.empty((B, S, D), np.float32)
    for c in range(N_CORES):
        b, h = c // 2, c % 2
        out[b, h * SQ : (h + 1) * SQ, :] = results[c]["out"]
    return out


def kernel(q, k, v, WQ, WK, WV, WO):
    nc = build_nc()
    in_maps = make_in_maps(q, k, v, WQ, WK, WV, WO)
    res = run_bass_kernel_spmd(nc, in_maps, core_ids=list(range(N_CORES)))
    return assemble(res.results)


if __name__ == "__main__":
    # quick self-run with random data
    rng = np.random.default_rng(0)
    q = rng.standard_normal((B, S, D)).astype(np.float32)
    k = rng.standard_normal((B, S, D)).astype(np.float32)
    v = rng.standard_normal((B, S, D)).astype(np.float32)
    WQ = rng.standard_normal((D, E)).astype(np.float32) * 0.08
    WK = rng.standard_normal((D, E)).astype(np.float32) * 0.08
    WV = rng.standard_normal((D, E)).astype(np.float32) * 0.08
    WO = rng.standard_normal((4 * E, D)).astype(np.float32) * 0.08
    out = kernel(q, k, v, WQ, WK, WV, WO)
    print("out", out.shape, out.dtype, np.abs(out).mean())
